# revision 7
# baseline (speedup 1.0000x reference)
"""Trainium2 Bass kernel for a 1-layer dense transformer LM (B=4, T=1024,
E=1024, H=16, HS=64, V=32000) sharded over 8 NeuronCores.

Sharding: tensor-parallel. Core c owns attention heads {2c, 2c+1} (model
dims 128c..128c+127), the matching ff_w row-slice for the FF partial sum
(combined with a single on-device AllReduce), and vocab columns
[4000c, 4000(c+1)) of the output projection.
"""

import sys

if "/opt/trn_rl_repo" not in sys.path:
    sys.path.insert(0, "/opt/trn_rl_repo")

import numpy as np

B, T, E, H, HS, V = 4, 1024, 1024, 16, 64, 32000
BT = B * T            # 4096 tokens
NC = 8                # cores
DH = 128              # model dims per core (2 heads x 64)
VS = V // NC          # 4000 vocab cols per core
VC = 500              # vocab chunk (8 chunks of 500 per core)
P = 128

_cache = {}
LAST_RESULT = None


def _build():
    import concourse.bacc as bacc
    import concourse.tile as tile
    from concourse import bass, mybir
    from concourse.masks import make_identity

    F32 = mybir.dt.float32
    F32R = mybir.dt.float32r
    I32 = mybir.dt.int32
    AL = mybir.AluOpType
    ACT = mybir.ActivationFunctionType

    nc = bacc.Bacc("TRN2", target_bir_lowering=False, debug=False,
                   num_devices=NC)

    # ---- I/O -----------------------------------------------------------
    xidx = nc.dram_tensor("xidx", [BT, 1], I32, kind="ExternalInput")
    temb = nc.dram_tensor("temb", [V, E], F32, kind="ExternalInput")
    pemb = nc.dram_tensor("pemb", [T, E], F32, kind="ExternalInput")
    wq = nc.dram_tensor("wq", [E, DH], F32R, kind="ExternalInput")
    wk = nc.dram_tensor("wk", [E, DH], F32R, kind="ExternalInput")
    wv = nc.dram_tensor("wv", [E, DH], F32R, kind="ExternalInput")
    ffw = nc.dram_tensor("ffw", [DH, E], F32R, kind="ExternalInput")
    ffb = nc.dram_tensor("ffb", [8, P], F32, kind="ExternalInput")
    fcw = nc.dram_tensor("fcw", [E, VS], F32R, kind="ExternalInput")
    fcb = nc.dram_tensor("fcb", [1, VS], F32, kind="ExternalInput")

    logits = nc.dram_tensor("logits", [BT, VS], F32, kind="ExternalOutput")
    esum = nc.dram_tensor("esum", [BT // P, P], F32, kind="ExternalOutput")

    # h^T staging (feature-major), FF partial sums + all-reduce result.
    hTd = nc.dram_tensor("hTd", [E, BT], F32R, kind="Internal")
    ffpart = nc.dram_tensor("ffpart", [E, BT], F32, kind="Internal")
    ffred = nc.dram_tensor("ffred", [E, BT], F32, kind="Internal",
                           addr_space="Shared")

    with tile.TileContext(nc) as tc, \
         nc.allow_low_precision("fp32r intermediates round to ~19-bit "
                                "mantissa; fine for this model"):
        with tc.tile_pool(name="const", bufs=1) as const_pool:
            ident = const_pool.tile([P, P], F32)
            make_identity(nc, ident[:])

            # ---- phase 1+2: embeddings -> hTd (feature-major, DRAM) ---
            with tc.tile_pool(name="posT", bufs=1) as posT_pool, \
                 tc.tile_pool(name="ph1", bufs=3) as ph1, \
                 tc.tile_pool(name="ps1", bufs=4, space="PSUM") as ps1:
                posT = posT_pool.tile([P, 8, T], F32)
                for g in range(8):
                    ptok = ph1.tile([P, E], F32, tag="ptok")
                    nc.sync.dma_start(ptok[:], pemb[g * P:(g + 1) * P, :])
                    for c in range(8):
                        ps = ps1.tile([P, P], F32, tag="ps")
                        nc.tensor.transpose(
                            ps[:], ptok[:, c * P:(c + 1) * P], ident[:])
                        nc.vector.tensor_copy(
                            posT[:, c, g * P:(g + 1) * P], ps[:])

                for g in range(32):
                    idx = ph1.tile([P, 1], I32, tag="idx")
                    nc.sync.dma_start(idx[:], xidx[g * P:(g + 1) * P, :])
                    htok = ph1.tile([P, E], F32, tag="htok")
                    nc.gpsimd.indirect_dma_start(
                        out=htok[:], out_offset=None,
                        in_=temb[:, :],
                        in_offset=bass.IndirectOffsetOnAxis(
                            ap=idx[:, :1], axis=0))
                    pcol = (g * P) % T
                    for c in range(8):
                        ps = ps1.tile([P, P], F32, tag="ps")
                        nc.tensor.transpose(
                            ps[:], htok[:, c * P:(c + 1) * P], ident[:])
                        hst = ph1.tile([P, P], F32R, tag="hst")
                        nc.vector.tensor_tensor(
                            out=hst[:], in0=ps[:],
                            in1=posT[:, c, pcol:pcol + P], op=AL.add)
                        nc.sync.dma_start(
                            hTd[c * P:(c + 1) * P, g * P:(g + 1) * P],
                            hst[:])

            # ---- phase 3: QKV projections (stream hTd once) -----------
            with tc.tile_pool(name="qkv", bufs=1) as qkv_pool:
                qT = qkv_pool.tile([P, BT], F32R)   # rows 0:64 h0, 64:128 h1
                kT = qkv_pool.tile([P, BT], F32R)
                vt = qkv_pool.tile([P, 32, 130], F32R)
                with tc.tile_pool(name="ph3w", bufs=1) as ph3w, \
                     tc.tile_pool(name="ph3", bufs=2) as ph3, \
                     tc.tile_pool(name="ps3", bufs=2, space="PSUM") as ps3:
                    wq_s = ph3w.tile([P, 8, DH], F32R, tag="wq")
                    wk_s = ph3w.tile([P, 8, DH], F32R, tag="wk")
                    wv_s = ph3w.tile([P, 8, DH], F32R, tag="wv")
                    nc.sync.dma_start(
                        wq_s[:], wq[:, :].rearrange("(k p) d -> p k d", p=P))
                    nc.sync.dma_start(
                        wk_s[:], wk[:, :].rearrange("(k p) d -> p k d", p=P))
                    nc.sync.dma_start(
                        wv_s[:], wv[:, :].rearrange("(k p) d -> p k d", p=P))
                    ones1 = ph3w.tile([P, 1], F32, tag="ones1")
                    nc.vector.memset(ones1[:], 1.0)
                    for g in range(32):
                        nc.vector.tensor_copy(vt[:, g, 64:65], ones1[:])
                        nc.vector.tensor_copy(vt[:, g, 129:130], ones1[:])

                    for n in range(8):
                        sl = slice(n * 512, (n + 1) * 512)
                        hk = ph3.tile([P, 8, 512], F32R, tag="hk")
                        nc.sync.dma_start(
                            hk[:],
                            hTd[:, sl].rearrange("(k p) t -> p k t", p=P))
                        psq = ps3.tile([P, 512], F32, tag="psq")
                        psk = ps3.tile([P, 512], F32, tag="psk")
                        for k in range(8):
                            nc.tensor.matmul(psq[:], lhsT=wq_s[:, k, :],
                                             rhs=hk[:, k, :],
                                             start=(k == 0), stop=(k == 7))
                        for k in range(8):
                            nc.tensor.matmul(psk[:], lhsT=wk_s[:, k, :],
                                             rhs=hk[:, k, :],
                                             start=(k == 0), stop=(k == 7))
                        nc.vector.tensor_copy(qT[:, sl], psq[:])
                        nc.vector.tensor_copy(kT[:, sl], psk[:])
                        for gg in range(4):
                            g = 4 * n + gg
                            psv = ps3.tile([P, DH], F32, tag="psv")
                            for k in range(8):
                                nc.tensor.matmul(
                                    psv[:],
                                    lhsT=hk[:, k, gg * P:(gg + 1) * P],
                                    rhs=wv_s[:, k, :],
                                    start=(k == 0), stop=(k == 7))
                            nc.vector.tensor_copy(vt[:, g, 0:64],
                                                  psv[:, 0:64])
                            nc.vector.tensor_copy(vt[:, g, 65:129],
                                                  psv[:, 64:128])

                # ---- phase 4: attention -------------------------------
                with tc.tile_pool(name="oT", bufs=1) as oT_pool:
                    oT = oT_pool.tile([P, BT], F32R)
                    with tc.tile_pool(name="amask", bufs=1) as amask_pool, \
                         tc.tile_pool(name="ph4", bufs=10) as ph4, \
                         tc.tile_pool(name="ph4b", bufs=3) as ph4b, \
                         tc.tile_pool(name="ps4s", bufs=3,
                                      space="PSUM") as ps4s, \
                         tc.tile_pool(name="ps4o", bufs=2,
                                      space="PSUM") as ps4o:
                        masks = amask_pool.tile([P, 4, 512], F32)
                        for ridx in range(4):
                            nc.gpsimd.memset(masks[:, ridx, :], 1.0)
                            # keep (=1.0) where f - p - ridx*128 >= 0,
                            # i.e. s = ridx*128+p <= t = f  (is_le is
                            # unsupported in walrus affine_select codegen)
                            nc.gpsimd.affine_select(
                                out=masks[:, ridx, :],
                                in_=masks[:, ridx, :],
                                compare_op=AL.is_ge, fill=0.0,
                                base=-(ridx * P), channel_multiplier=-1,
                                pattern=[[1, 512]])

                        for hh in range(2):
                            hsl = slice(64 * hh, 64 * hh + 64)
                            vsl = slice(65 * hh, 65 * hh + 65)
                            for b in range(4):
                                for j in range(2):
                                    t0 = b * 1024 + j * 512
                                    i_max = 4 * j + 3
                                    exps = []
                                    for i in range(i_max + 1):
                                        s0 = b * 1024 + i * P
                                        pss = ps4s.tile([P, 512], F32,
                                                        tag="pss")
                                        nc.tensor.matmul(
                                            pss[:],
                                            lhsT=kT[hsl, s0:s0 + P],
                                            rhs=qT[hsl, t0:t0 + 512],
                                            start=True, stop=True)
                                        ex = ph4.tile([P, 512], F32R,
                                                      tag="ex")
                                        nc.scalar.activation(
                                            ex[:], pss[:], ACT.Exp,
                                            scale=float(HS ** 0.5))
                                        r = i * P - j * 512
                                        if r >= 0:
                                            nc.vector.tensor_tensor(
                                                out=ex[:], in0=ex[:],
                                                in1=masks[:, r // P, :],
                                                op=AL.mult)
                                        exps.append(ex)
                                    pso = ps4o.tile([65, 512], F32,
                                                    tag="pso")
                                    for i in range(i_max + 1):
                                        g = b * 8 + i
                                        nc.tensor.matmul(
                                            pso[:], lhsT=vt[:, g, vsl],
                                            rhs=exps[i][:],
                                            start=(i == 0),
                                            stop=(i == i_max))
                                    rec = ph4b.tile([1, 512], F32R,
                                                    tag="rec")
                                    nc.vector.reciprocal(rec[:],
                                                         pso[64:65, :])
                                    rbc = ph4b.tile([64, 512], F32R,
                                                    tag="rbc")
                                    nc.gpsimd.partition_broadcast(
                                        rbc[:], rec[:1, :])
                                    nc.vector.tensor_tensor(
                                        out=oT[hsl, t0:t0 + 512],
                                        in0=pso[0:64, :], in1=rbc[:],
                                        op=AL.mult)

                    # ---- phase 5: FF partial + AllReduce --------------
                    with tc.tile_pool(name="ph5", bufs=3) as ph5, \
                         tc.tile_pool(name="ps5", bufs=4,
                                      space="PSUM") as ps5:
                        ffw_s = ph5.tile([P, E], F32R, tag="ffw")
                        nc.sync.dma_start(ffw_s[:], ffw[:, :])
                        for m in range(8):
                            for n in range(8):
                                ps = ps5.tile([P, 512], F32, tag="ps")
                                nc.tensor.matmul(
                                    ps[:],
                                    lhsT=ffw_s[:, m * P:(m + 1) * P],
                                    rhs=oT[:, n * 512:(n + 1) * 512],
                                    start=True, stop=True)
                                st = ph5.tile([P, 512], F32, tag="st")
                                nc.vector.tensor_copy(st[:], ps[:])
                                nc.sync.dma_start(
                                    ffpart[m * P:(m + 1) * P,
                                           n * 512:(n + 1) * 512], st[:])

            nc.gpsimd.collective_compute(
                "AllReduce", AL.add,
                replica_groups=[list(range(NC))],
                ins=[ffpart[:, :].opt()],
                outs=[ffred[:, :].opt()],
            )

            # ---- phase 6: relu(ff) + fc + exp-sums --------------------
            with tc.tile_pool(name="ffT", bufs=1) as ffT_pool, \
                 tc.tile_pool(name="ph6w", bufs=2) as ph6w, \
                 tc.tile_pool(name="ph6", bufs=4) as ph6, \
                 tc.tile_pool(name="ph6s", bufs=1) as ph6s, \
                 tc.tile_pool(name="ps6", bufs=8, space="PSUM") as ps6:
                ffT = ffT_pool.tile([P, 8, BT], F32R)
                ffb_s = ph6s.tile([P, 8], F32)
                nc.sync.dma_start(ffb_s[:], ffb[:, :].rearrange("m p -> p m"))
                fcb_s = ph6s.tile([1, VS], F32)
                nc.sync.dma_start(fcb_s[:], fcb[:, :])
                es = ph6s.tile([P, 32, 8], F32)
                esf = ph6s.tile([P, 32], F32)

                for k in range(8):
                    nc.sync.dma_start(ffT[:, k, :].bitcast(F32),
                                      ffred[k * P:(k + 1) * P, :])
                    nc.scalar.activation(ffT[:, k, :],
                                         ffT[:, k, :].bitcast(F32),
                                         ACT.Relu, bias=ffb_s[:, k:k + 1])

                for n in range(8):
                    nsl = slice(n * VC, (n + 1) * VC)
                    fcw_t = ph6w.tile([P, 8, VC], F32R, tag="fcw")
                    nc.sync.dma_start(
                        fcw_t[:],
                        fcw[:, nsl].rearrange("(k p) v -> p k v", p=P))
                    bbc = ph6w.tile([P, VC], F32, tag="bbc")
                    nc.gpsimd.partition_broadcast(bbc[:], fcb_s[:1, nsl])
                    for m in range(32):
                        ps = ps6.tile([P, VC], F32, tag="ps")
                        for k in range(8):
                            nc.tensor.matmul(
                                ps[:], lhsT=ffT[:, k, m * P:(m + 1) * P],
                                rhs=fcw_t[:, k, :],
                                start=(k == 0), stop=(k == 7))
                        lg = ph6.tile([P, VC], F32, tag="lg")
                        nc.vector.tensor_tensor(lg[:], ps[:], bbc[:],
                                                op=AL.add)
                        nc.sync.dma_start(
                            logits[m * P:(m + 1) * P, nsl], lg[:])
                        ex = ph6.tile([P, VC], F32, tag="exl")
                        nc.scalar.activation(ex[:], lg[:], ACT.Exp,
                                             accum_out=es[:, m, n:n + 1])

                for m in range(32):
                    nc.vector.reduce_sum(out=esf[:, m:m + 1],
                                         in_=es[:, m, :],
                                         axis=mybir.AxisListType.X)
                nc.sync.dma_start(
                    esum[:, :].rearrange("m p -> p m"), esf[:])

    nc.compile()
    return nc


def _get_nc():
    if "nc" not in _cache:
        _cache["nc"] = _build()
    return _cache["nc"]


def kernel(**inputs):
    global LAST_RESULT
    from concourse.bass_utils import run_bass_kernel_spmd

    x = np.asarray(inputs["x"]).reshape(BT, 1).astype(np.int32)
    y = np.asarray(inputs["y"]).reshape(BT).astype(np.int64)
    tok_emb = np.ascontiguousarray(np.asarray(inputs["tok_emb"], np.float32))
    pos_emb = np.ascontiguousarray(np.asarray(inputs["pos_emb"], np.float32))
    Wq = np.asarray(inputs["Wq"], np.float32)
    Wk = np.asarray(inputs["Wk"], np.float32)
    Wv = np.asarray(inputs["Wv"], np.float32)
    ff_w = np.asarray(inputs["ff_w"], np.float32)
    ff_b = np.asarray(inputs["ff_b"], np.float32)
    fc_w = np.asarray(inputs["fc_w"], np.float32)
    fc_b = np.asarray(inputs["fc_b"], np.float32)

    def wslice(W, c):
        # heads {2c, 2c+1}: [2, E, HS] -> [E, 2*HS]
        return np.ascontiguousarray(
            W[2 * c:2 * c + 2].transpose(1, 0, 2).reshape(E, DH))

    in_maps = []
    for c in range(NC):
        in_maps.append({
            "xidx": x,
            "temb": tok_emb,
            "pemb": pos_emb,
            "wq": wslice(Wq, c),
            "wk": wslice(Wk, c),
            "wv": wslice(Wv, c),
            "ffw": np.ascontiguousarray(ff_w[c * DH:(c + 1) * DH, :]),
            "ffb": np.ascontiguousarray(ff_b.reshape(8, P)),
            "fcw": np.ascontiguousarray(fc_w[:, c * VS:(c + 1) * VS]),
            "fcb": np.ascontiguousarray(fc_b[c * VS:(c + 1) * VS])[None, :],
        })

    nc = _get_nc()
    res = run_bass_kernel_spmd(nc, in_maps, core_ids=list(range(NC)))
    LAST_RESULT = res

    logits_full = np.concatenate(
        [res.results[c]["logits"] for c in range(NC)], axis=1)
    S = np.zeros(BT, np.float64)
    for c in range(NC):
        S += res.results[c]["esum"].reshape(BT).astype(np.float64)
    lse = np.log(S)
    tgt = logits_full[np.arange(BT), y].astype(np.float64)
    loss = np.float32(np.mean(lse - tgt))
    return logits_full, loss


# revision 9
# speedup vs baseline: 1.1672x; 1.1672x over previous
"""Trainium2 Bass kernel for a 1-layer dense transformer LM (B=4, T=1024,
E=1024, H=16, HS=64, V=32000) sharded over 8 NeuronCores.

Sharding: tensor-parallel. Core c owns attention heads {2c, 2c+1} (model
dims 128c..128c+127), the matching ff_w row-slice for the FF partial sum
(combined with a single on-device AllReduce), and vocab columns
[4000c, 4000(c+1)) of the output projection.
"""

import sys

if "/opt/trn_rl_repo" not in sys.path:
    sys.path.insert(0, "/opt/trn_rl_repo")

import numpy as np

B, T, E, H, HS, V = 4, 1024, 1024, 16, 64, 32000
BT = B * T            # 4096 tokens
NC = 8                # cores
DH = 128              # model dims per core (2 heads x 64)
VS = V // NC          # 4000 vocab cols per core
VC = 500              # vocab chunk (8 chunks of 500 per core)
P = 128

_cache = {}
LAST_RESULT = None


def _build():
    import concourse.bacc as bacc
    import concourse.tile as tile
    from concourse import bass, mybir
    from concourse.masks import make_identity

    F32 = mybir.dt.float32
    F32R = mybir.dt.float32r
    I32 = mybir.dt.int32
    AL = mybir.AluOpType
    ACT = mybir.ActivationFunctionType

    nc = bacc.Bacc("TRN2", target_bir_lowering=False, debug=False,
                   num_devices=NC)

    # ---- I/O -----------------------------------------------------------
    xidx = nc.dram_tensor("xidx", [BT, 1], I32, kind="ExternalInput")
    temb = nc.dram_tensor("temb", [V, E], F32, kind="ExternalInput")
    pemb = nc.dram_tensor("pemb", [T, E], F32, kind="ExternalInput")
    wq = nc.dram_tensor("wq", [E, DH], F32R, kind="ExternalInput")
    wk = nc.dram_tensor("wk", [E, DH], F32R, kind="ExternalInput")
    wv = nc.dram_tensor("wv", [E, DH], F32R, kind="ExternalInput")
    ffw = nc.dram_tensor("ffw", [DH, E], F32R, kind="ExternalInput")
    ffb = nc.dram_tensor("ffb", [8, P], F32, kind="ExternalInput")
    fcw = nc.dram_tensor("fcw", [E, VS], F32R, kind="ExternalInput")
    fcb = nc.dram_tensor("fcb", [1, VS], F32, kind="ExternalInput")

    logits = nc.dram_tensor("logits", [BT, VS], F32, kind="ExternalOutput")
    esum = nc.dram_tensor("esum", [BT // P, P], F32, kind="ExternalOutput")

    # h^T staging (feature-major), FF partial sums + all-reduce result.
    hTd = nc.dram_tensor("hTd", [E, BT], F32R, kind="Internal")
    ffpart = nc.dram_tensor("ffpart", [E, BT], F32, kind="Internal")
    ffred = nc.dram_tensor("ffred", [E, BT], F32, kind="Internal",
                           addr_space="Shared")

    with tile.TileContext(nc) as tc, \
         nc.allow_low_precision("fp32r intermediates round to ~19-bit "
                                "mantissa; fine for this model"):
        with tc.tile_pool(name="const", bufs=1) as const_pool:
            ident = const_pool.tile([P, P], F32)
            make_identity(nc, ident[:])

            # ---- phases 1-3 fused: embeddings -> h^T chunks -> QKV ----
            # For each 512-token chunk: gather 4x128 token embeddings,
            # PE-transpose to feature-major, add pos. emb. into an SBUF
            # chunk hk, then immediately run the QKV matmuls on it.
            with tc.tile_pool(name="qkv", bufs=1) as qkv_pool:
                qT = qkv_pool.tile([P, BT], F32R)   # rows 0:64 h0, 64:128 h1
                kT = qkv_pool.tile([P, BT], F32R)
                vt = qkv_pool.tile([P, 32, 130], F32R)
                with tc.tile_pool(name="posT", bufs=1) as posT_pool, \
                     tc.tile_pool(name="ph3w", bufs=1) as ph3w, \
                     tc.tile_pool(name="ph1", bufs=6) as ph1, \
                     tc.tile_pool(name="ph3", bufs=2) as ph3, \
                     tc.tile_pool(name="psA", bufs=2, space="PSUM") as psA, \
                     tc.tile_pool(name="ps3", bufs=2, space="PSUM") as ps3:
                    posT = posT_pool.tile([P, 8, T], F32)
                    for g in range(8):
                        ptok = ph1.tile([P, E], F32, tag="ptok")
                        nc.sync.dma_start(ptok[:], pemb[g * P:(g + 1) * P, :])
                        for c in range(8):
                            ps = psA.tile([P, 512], F32, tag="psb")
                            nc.tensor.transpose(
                                ps[:, (g % 4) * P:(g % 4 + 1) * P],
                                ptok[:, c * P:(c + 1) * P], ident[:])
                            nc.vector.tensor_copy(
                                posT[:, c, g * P:(g + 1) * P],
                                ps[:, (g % 4) * P:(g % 4 + 1) * P])

                    wq_s = ph3w.tile([P, 8, DH], F32R, tag="wq")
                    wk_s = ph3w.tile([P, 8, DH], F32R, tag="wk")
                    wv_s = ph3w.tile([P, 8, DH], F32R, tag="wv")
                    nc.sync.dma_start(
                        wq_s[:], wq[:, :].rearrange("(k p) d -> p k d", p=P))
                    nc.sync.dma_start(
                        wk_s[:], wk[:, :].rearrange("(k p) d -> p k d", p=P))
                    nc.sync.dma_start(
                        wv_s[:], wv[:, :].rearrange("(k p) d -> p k d", p=P))
                    ones1 = ph3w.tile([P, 1], F32, tag="ones1")
                    nc.vector.memset(ones1[:], 1.0)
                    for g in range(32):
                        nc.vector.tensor_copy(vt[:, g, 64:65], ones1[:])
                        nc.vector.tensor_copy(vt[:, g, 129:130], ones1[:])

                    for n in range(8):
                        sl = slice(n * 512, (n + 1) * 512)
                        hts = []
                        for gg in range(4):
                            g = 4 * n + gg
                            idx = ph1.tile([P, 1], I32, tag="idx")
                            nc.sync.dma_start(idx[:],
                                              xidx[g * P:(g + 1) * P, :])
                            htok = ph1.tile([P, E], F32, tag="htok")
                            nc.gpsimd.indirect_dma_start(
                                out=htok[:], out_offset=None,
                                in_=temb[:, :],
                                in_offset=bass.IndirectOffsetOnAxis(
                                    ap=idx[:, :1], axis=0))
                            hts.append(htok)
                        hk = ph3.tile([P, 8, 512], F32R, tag="hk")
                        pc0 = (n % 2) * 512
                        for c in range(8):
                            psb = psA.tile([P, 512], F32, tag="psb")
                            for gg in range(4):
                                nc.tensor.transpose(
                                    psb[:, gg * P:(gg + 1) * P],
                                    hts[gg][:, c * P:(c + 1) * P], ident[:])
                            nc.vector.tensor_tensor(
                                out=hk[:, c, :], in0=psb[:],
                                in1=posT[:, c, pc0:pc0 + 512], op=AL.add)
                        psq = ps3.tile([P, 512], F32, tag="psq")
                        psk = ps3.tile([P, 512], F32, tag="psk")
                        psv = ps3.tile([P, 512], F32, tag="psv")
                        for k in range(8):
                            nc.tensor.matmul(psq[:], lhsT=wq_s[:, k, :],
                                             rhs=hk[:, k, :],
                                             start=(k == 0), stop=(k == 7))
                        for k in range(8):
                            nc.tensor.matmul(psk[:], lhsT=wk_s[:, k, :],
                                             rhs=hk[:, k, :],
                                             start=(k == 0), stop=(k == 7))
                        for k in range(8):
                            nc.tensor.matmul(psv[:], lhsT=wv_s[:, k, :],
                                             rhs=hk[:, k, :],
                                             start=(k == 0), stop=(k == 7))
                        nc.vector.tensor_copy(qT[:, sl], psq[:])
                        nc.vector.tensor_copy(kT[:, sl], psk[:])
                        # v back to token-major via PE transpose
                        vTs = ph3.tile([P, 512], F32, tag="vTs")
                        nc.vector.tensor_copy(vTs[:], psv[:])
                        psb = psA.tile([P, 512], F32, tag="psb")
                        for gg in range(4):
                            nc.tensor.transpose(
                                psb[:, gg * P:(gg + 1) * P],
                                vTs[:, gg * P:(gg + 1) * P], ident[:])
                        pv = psb[:].rearrange("p (g d) -> p g d", g=4)
                        nc.vector.tensor_copy(vt[:, 4 * n:4 * n + 4, 0:64],
                                              pv[:, :, 0:64])
                        nc.vector.tensor_copy(vt[:, 4 * n:4 * n + 4, 65:129],
                                              pv[:, :, 64:128])

                # ---- phase 4: attention -------------------------------
                with tc.tile_pool(name="oT", bufs=1) as oT_pool:
                    oT = oT_pool.tile([P, BT], F32R)
                    with tc.tile_pool(name="amask", bufs=1) as amask_pool, \
                         tc.tile_pool(name="ph4", bufs=10) as ph4, \
                         tc.tile_pool(name="ph4b", bufs=3) as ph4b, \
                         tc.tile_pool(name="ps4s", bufs=3,
                                      space="PSUM") as ps4s, \
                         tc.tile_pool(name="ps4o", bufs=2,
                                      space="PSUM") as ps4o:
                        masks = amask_pool.tile([P, 4, 512], F32)
                        for ridx in range(4):
                            nc.gpsimd.memset(masks[:, ridx, :], 1.0)
                            # keep (=1.0) where f - p - ridx*128 >= 0,
                            # i.e. s = ridx*128+p <= t = f  (is_le is
                            # unsupported in walrus affine_select codegen)
                            nc.gpsimd.affine_select(
                                out=masks[:, ridx, :],
                                in_=masks[:, ridx, :],
                                compare_op=AL.is_ge, fill=0.0,
                                base=-(ridx * P), channel_multiplier=-1,
                                pattern=[[1, 512]])

                        for hh in range(2):
                            hsl = slice(64 * hh, 64 * hh + 64)
                            vsl = slice(65 * hh, 65 * hh + 65)
                            for b in range(4):
                                for j in range(2):
                                    t0 = b * 1024 + j * 512
                                    i_max = 4 * j + 3
                                    exps = []
                                    for i in range(i_max + 1):
                                        s0 = b * 1024 + i * P
                                        pss = ps4s.tile([P, 512], F32,
                                                        tag="pss")
                                        nc.tensor.matmul(
                                            pss[:],
                                            lhsT=kT[hsl, s0:s0 + P],
                                            rhs=qT[hsl, t0:t0 + 512],
                                            start=True, stop=True)
                                        ex = ph4.tile([P, 512], F32R,
                                                      tag="ex")
                                        nc.scalar.activation(
                                            ex[:], pss[:], ACT.Exp,
                                            scale=float(HS ** 0.5))
                                        r = i * P - j * 512
                                        if r >= 0:
                                            nc.vector.tensor_tensor(
                                                out=ex[:], in0=ex[:],
                                                in1=masks[:, r // P, :],
                                                op=AL.mult)
                                        exps.append(ex)
                                    pso = ps4o.tile([65, 512], F32,
                                                    tag="pso")
                                    for i in range(i_max + 1):
                                        g = b * 8 + i
                                        nc.tensor.matmul(
                                            pso[:], lhsT=vt[:, g, vsl],
                                            rhs=exps[i][:],
                                            start=(i == 0),
                                            stop=(i == i_max))
                                    rec = ph4b.tile([1, 512], F32R,
                                                    tag="rec")
                                    nc.vector.reciprocal(rec[:],
                                                         pso[64:65, :])
                                    rbc = ph4b.tile([64, 512], F32R,
                                                    tag="rbc")
                                    nc.gpsimd.partition_broadcast(
                                        rbc[:], rec[:1, :])
                                    nc.vector.tensor_tensor(
                                        out=oT[hsl, t0:t0 + 512],
                                        in0=pso[0:64, :], in1=rbc[:],
                                        op=AL.mult)

                    # ---- phase 5: FF partial + per-slice AllReduce ----
                    # One AllReduce per d'-slice, fired as soon as that
                    # slice's partials are in DRAM, so the reduction
                    # pipeline overlaps the start of the fc below.
                    with tc.tile_pool(name="ph5", bufs=3) as ph5, \
                         tc.tile_pool(name="ps5", bufs=4,
                                      space="PSUM") as ps5:
                        ffw_s = ph5.tile([P, E], F32R, tag="ffw")
                        nc.sync.dma_start(ffw_s[:], ffw[:, :])
                        for m in range(8):
                            for n in range(8):
                                ps = ps5.tile([P, 512], F32, tag="ps")
                                nc.tensor.matmul(
                                    ps[:],
                                    lhsT=ffw_s[:, m * P:(m + 1) * P],
                                    rhs=oT[:, n * 512:(n + 1) * 512],
                                    start=True, stop=True)
                                st = ph5.tile([P, 512], F32, tag="st")
                                nc.vector.tensor_copy(st[:], ps[:])
                                nc.sync.dma_start(
                                    ffpart[m * P:(m + 1) * P,
                                           n * 512:(n + 1) * 512], st[:])
                            nc.gpsimd.collective_compute(
                                "AllReduce", AL.add,
                                replica_groups=[list(range(NC))],
                                ins=[ffpart[m * P:(m + 1) * P, :].opt()],
                                outs=[ffred[m * P:(m + 1) * P, :].opt()],
                            )

            # ---- phase 6: relu(ff) + fc + exp-sums --------------------
            with tc.tile_pool(name="ffT", bufs=1) as ffT_pool, \
                 tc.tile_pool(name="ph6w", bufs=2) as ph6w, \
                 tc.tile_pool(name="ph6", bufs=4) as ph6, \
                 tc.tile_pool(name="ph6s", bufs=1) as ph6s, \
                 tc.tile_pool(name="ps6", bufs=8, space="PSUM") as ps6:
                ffT = ffT_pool.tile([P, 8, BT], F32R)
                ffb_s = ph6s.tile([P, 8], F32)
                nc.sync.dma_start(ffb_s[:], ffb[:, :].rearrange("m p -> p m"))
                fcb_s = ph6s.tile([1, VS], F32)
                nc.sync.dma_start(fcb_s[:], fcb[:, :])
                es = ph6s.tile([P, 32, 8], F32)
                esf = ph6s.tile([P, 32], F32)

                for k in range(8):
                    nc.sync.dma_start(ffT[:, k, :].bitcast(F32),
                                      ffred[k * P:(k + 1) * P, :])
                    nc.scalar.activation(ffT[:, k, :],
                                         ffT[:, k, :].bitcast(F32),
                                         ACT.Relu, bias=ffb_s[:, k:k + 1])

                for n in range(8):
                    nsl = slice(n * VC, (n + 1) * VC)
                    fcw_t = ph6w.tile([P, 8, VC], F32R, tag="fcw")
                    nc.sync.dma_start(
                        fcw_t[:],
                        fcw[:, nsl].rearrange("(k p) v -> p k v", p=P))
                    bbc = ph6w.tile([P, VC], F32, tag="bbc")
                    nc.gpsimd.partition_broadcast(bbc[:], fcb_s[:1, nsl])
                    for m in range(32):
                        ps = ps6.tile([P, VC], F32, tag="ps")
                        for k in range(8):
                            nc.tensor.matmul(
                                ps[:], lhsT=ffT[:, k, m * P:(m + 1) * P],
                                rhs=fcw_t[:, k, :],
                                start=(k == 0), stop=(k == 7))
                        lg = ph6.tile([P, VC], F32, tag="lg")
                        nc.vector.tensor_tensor(lg[:], ps[:], bbc[:],
                                                op=AL.add)
                        nc.sync.dma_start(
                            logits[m * P:(m + 1) * P, nsl], lg[:])
                        ex = ph6.tile([P, VC], F32, tag="exl")
                        nc.scalar.activation(ex[:], lg[:], ACT.Exp,
                                             accum_out=es[:, m, n:n + 1])

                for m in range(32):
                    nc.vector.reduce_sum(out=esf[:, m:m + 1],
                                         in_=es[:, m, :],
                                         axis=mybir.AxisListType.X)
                nc.sync.dma_start(
                    esum[:, :].rearrange("m p -> p m"), esf[:])

    nc.compile()
    return nc


def _get_nc():
    if "nc" not in _cache:
        _cache["nc"] = _build()
    return _cache["nc"]


def kernel(**inputs):
    global LAST_RESULT
    from concourse.bass_utils import run_bass_kernel_spmd

    x = np.asarray(inputs["x"]).reshape(BT, 1).astype(np.int32)
    y = np.asarray(inputs["y"]).reshape(BT).astype(np.int64)
    tok_emb = np.ascontiguousarray(np.asarray(inputs["tok_emb"], np.float32))
    pos_emb = np.ascontiguousarray(np.asarray(inputs["pos_emb"], np.float32))
    Wq = np.asarray(inputs["Wq"], np.float32)
    Wk = np.asarray(inputs["Wk"], np.float32)
    Wv = np.asarray(inputs["Wv"], np.float32)
    ff_w = np.asarray(inputs["ff_w"], np.float32)
    ff_b = np.asarray(inputs["ff_b"], np.float32)
    fc_w = np.asarray(inputs["fc_w"], np.float32)
    fc_b = np.asarray(inputs["fc_b"], np.float32)

    def wslice(W, c):
        # heads {2c, 2c+1}: [2, E, HS] -> [E, 2*HS]
        return np.ascontiguousarray(
            W[2 * c:2 * c + 2].transpose(1, 0, 2).reshape(E, DH))

    in_maps = []
    for c in range(NC):
        in_maps.append({
            "xidx": x,
            "temb": tok_emb,
            "pemb": pos_emb,
            "wq": wslice(Wq, c),
            "wk": wslice(Wk, c),
            "wv": wslice(Wv, c),
            "ffw": np.ascontiguousarray(ff_w[c * DH:(c + 1) * DH, :]),
            "ffb": np.ascontiguousarray(ff_b.reshape(8, P)),
            "fcw": np.ascontiguousarray(fc_w[:, c * VS:(c + 1) * VS]),
            "fcb": np.ascontiguousarray(fc_b[c * VS:(c + 1) * VS])[None, :],
        })

    nc = _get_nc()
    res = run_bass_kernel_spmd(nc, in_maps, core_ids=list(range(NC)))
    LAST_RESULT = res

    logits_full = np.concatenate(
        [res.results[c]["logits"] for c in range(NC)], axis=1)
    S = np.zeros(BT, np.float64)
    for c in range(NC):
        S += res.results[c]["esum"].reshape(BT).astype(np.float64)
    lse = np.log(S)
    tgt = logits_full[np.arange(BT), y].astype(np.float64)
    loss = np.float32(np.mean(lse - tgt))
    return logits_full, loss


# revision 10
# speedup vs baseline: 1.3018x; 1.1153x over previous
"""Trainium2 Bass kernel for a 1-layer dense transformer LM (B=4, T=1024,
E=1024, H=16, HS=64, V=32000) sharded over 8 NeuronCores.

Sharding: tensor-parallel. Core c owns attention heads {2c, 2c+1} (model
dims 128c..128c+127), the matching ff_w row-slice for the FF partial sum
(combined with a single on-device AllReduce), and vocab columns
[4000c, 4000(c+1)) of the output projection.
"""

import sys

if "/opt/trn_rl_repo" not in sys.path:
    sys.path.insert(0, "/opt/trn_rl_repo")

import numpy as np

B, T, E, H, HS, V = 4, 1024, 1024, 16, 64, 32000
BT = B * T            # 4096 tokens
NC = 8                # cores
DH = 128              # model dims per core (2 heads x 64)
VS = V // NC          # 4000 vocab cols per core
VC = 500              # vocab chunk (8 chunks of 500 per core)
P = 128

_cache = {}
LAST_RESULT = None


def _build():
    import concourse.bacc as bacc
    import concourse.tile as tile
    from concourse import bass, mybir
    from concourse.masks import make_identity

    F32 = mybir.dt.float32
    F32R = mybir.dt.float32r
    I32 = mybir.dt.int32
    AL = mybir.AluOpType
    ACT = mybir.ActivationFunctionType

    nc = bacc.Bacc("TRN2", target_bir_lowering=False, debug=False,
                   num_devices=NC)

    # ---- I/O -----------------------------------------------------------
    xidx = nc.dram_tensor("xidx", [BT, 1], I32, kind="ExternalInput")
    temb = nc.dram_tensor("temb", [V, E], F32, kind="ExternalInput")
    pemb = nc.dram_tensor("pemb", [T, E], F32, kind="ExternalInput")
    wq = nc.dram_tensor("wq", [E, DH], F32R, kind="ExternalInput")
    wk = nc.dram_tensor("wk", [E, DH], F32R, kind="ExternalInput")
    wv = nc.dram_tensor("wv", [E, DH], F32R, kind="ExternalInput")
    ffw = nc.dram_tensor("ffw", [DH, E], F32R, kind="ExternalInput")
    ffb = nc.dram_tensor("ffb", [8, P], F32, kind="ExternalInput")
    fcw = nc.dram_tensor("fcw", [E, VS], F32R, kind="ExternalInput")
    fcb = nc.dram_tensor("fcb", [1, VS], F32, kind="ExternalInput")

    logits = nc.dram_tensor("logits", [BT, VS], F32, kind="ExternalOutput")
    esum = nc.dram_tensor("esum", [BT // P, P], F32, kind="ExternalOutput")

    # h^T staging (feature-major), FF partial sums + all-reduce result.
    hTd = nc.dram_tensor("hTd", [E, BT], F32R, kind="Internal")
    ffpart = nc.dram_tensor("ffpart", [8, E, 512], F32, kind="Internal")
    ffred = nc.dram_tensor("ffred", [8, E, 512], F32, kind="Internal",
                           addr_space="Shared")

    with tile.TileContext(nc) as tc, \
         nc.allow_low_precision("fp32r intermediates round to ~19-bit "
                                "mantissa; fine for this model"):
        with tc.tile_pool(name="const", bufs=1) as const_pool:
            ident = const_pool.tile([P, P], F32)
            make_identity(nc, ident[:])

            # ---- phases 1-3 fused: embeddings -> h^T chunks -> QKV ----
            # For each 512-token chunk: gather 4x128 token embeddings,
            # PE-transpose to feature-major, add pos. emb. into an SBUF
            # chunk hk, then immediately run the QKV matmuls on it.
            with tc.tile_pool(name="qkv", bufs=1) as qkv_pool:
                qT = qkv_pool.tile([P, BT], F32R)   # rows 0:64 h0, 64:128 h1
                kT = qkv_pool.tile([P, BT], F32R)
                vt = qkv_pool.tile([P, 32, 130], F32R)
                with tc.tile_pool(name="posT", bufs=1) as posT_pool, \
                     tc.tile_pool(name="ph3w", bufs=1) as ph3w, \
                     tc.tile_pool(name="ph1", bufs=6) as ph1, \
                     tc.tile_pool(name="ph3", bufs=2) as ph3, \
                     tc.tile_pool(name="psA", bufs=2, space="PSUM") as psA, \
                     tc.tile_pool(name="ps3", bufs=2, space="PSUM") as ps3:
                    posT = posT_pool.tile([P, 8, T], F32)
                    for g in range(8):
                        ptok = ph1.tile([P, E], F32, tag="ptok")
                        nc.sync.dma_start(ptok[:], pemb[g * P:(g + 1) * P, :])
                        for c in range(8):
                            ps = psA.tile([P, 512], F32, tag="psb")
                            nc.tensor.transpose(
                                ps[:, (g % 4) * P:(g % 4 + 1) * P],
                                ptok[:, c * P:(c + 1) * P], ident[:])
                            nc.vector.tensor_copy(
                                posT[:, c, g * P:(g + 1) * P],
                                ps[:, (g % 4) * P:(g % 4 + 1) * P])

                    wq_s = ph3w.tile([P, 8, DH], F32R, tag="wq")
                    wk_s = ph3w.tile([P, 8, DH], F32R, tag="wk")
                    wv_s = ph3w.tile([P, 8, DH], F32R, tag="wv")
                    nc.sync.dma_start(
                        wq_s[:], wq[:, :].rearrange("(k p) d -> p k d", p=P))
                    nc.sync.dma_start(
                        wk_s[:], wk[:, :].rearrange("(k p) d -> p k d", p=P))
                    nc.sync.dma_start(
                        wv_s[:], wv[:, :].rearrange("(k p) d -> p k d", p=P))
                    ones1 = ph3w.tile([P, 1], F32, tag="ones1")
                    nc.vector.memset(ones1[:], 1.0)
                    for g in range(32):
                        nc.vector.tensor_copy(vt[:, g, 64:65], ones1[:])
                        nc.vector.tensor_copy(vt[:, g, 129:130], ones1[:])

                    for n in range(8):
                        sl = slice(n * 512, (n + 1) * 512)
                        hts = []
                        for gg in range(4):
                            g = 4 * n + gg
                            idx = ph1.tile([P, 1], I32, tag="idx")
                            nc.sync.dma_start(idx[:],
                                              xidx[g * P:(g + 1) * P, :])
                            htok = ph1.tile([P, E], F32, tag="htok")
                            nc.gpsimd.indirect_dma_start(
                                out=htok[:], out_offset=None,
                                in_=temb[:, :],
                                in_offset=bass.IndirectOffsetOnAxis(
                                    ap=idx[:, :1], axis=0))
                            hts.append(htok)
                        hk = ph3.tile([P, 8, 512], F32R, tag="hk")
                        pc0 = (n % 2) * 512
                        for c in range(8):
                            psb = psA.tile([P, 512], F32, tag="psb")
                            for gg in range(4):
                                nc.tensor.transpose(
                                    psb[:, gg * P:(gg + 1) * P],
                                    hts[gg][:, c * P:(c + 1) * P], ident[:])
                            nc.vector.tensor_tensor(
                                out=hk[:, c, :], in0=psb[:],
                                in1=posT[:, c, pc0:pc0 + 512], op=AL.add)
                        psq = ps3.tile([P, 512], F32, tag="psq")
                        psk = ps3.tile([P, 512], F32, tag="psk")
                        psv = ps3.tile([P, 512], F32, tag="psv")
                        for k in range(8):
                            nc.tensor.matmul(psq[:], lhsT=wq_s[:, k, :],
                                             rhs=hk[:, k, :],
                                             start=(k == 0), stop=(k == 7))
                        for k in range(8):
                            nc.tensor.matmul(psk[:], lhsT=wk_s[:, k, :],
                                             rhs=hk[:, k, :],
                                             start=(k == 0), stop=(k == 7))
                        for k in range(8):
                            nc.tensor.matmul(psv[:], lhsT=wv_s[:, k, :],
                                             rhs=hk[:, k, :],
                                             start=(k == 0), stop=(k == 7))
                        nc.vector.tensor_copy(qT[:, sl], psq[:])
                        nc.vector.tensor_copy(kT[:, sl], psk[:])
                        # v back to token-major via PE transpose
                        vTs = ph3.tile([P, 512], F32, tag="vTs")
                        nc.vector.tensor_copy(vTs[:], psv[:])
                        psb = psA.tile([P, 512], F32, tag="psb")
                        for gg in range(4):
                            nc.tensor.transpose(
                                psb[:, gg * P:(gg + 1) * P],
                                vTs[:, gg * P:(gg + 1) * P], ident[:])
                        pv = psb[:].rearrange("p (g d) -> p g d", g=4)
                        nc.vector.tensor_copy(vt[:, 4 * n:4 * n + 4, 0:64],
                                              pv[:, :, 0:64])
                        nc.vector.tensor_copy(vt[:, 4 * n:4 * n + 4, 65:129],
                                              pv[:, :, 64:128])

                # ---- phase 4: attention -------------------------------
                with tc.tile_pool(name="oT", bufs=1) as oT_pool:
                    oT = oT_pool.tile([P, BT], F32R)
                    with tc.tile_pool(name="amask", bufs=1) as amask_pool, \
                         tc.tile_pool(name="ph4", bufs=10) as ph4, \
                         tc.tile_pool(name="ph4b", bufs=3) as ph4b, \
                         tc.tile_pool(name="ps4s", bufs=3,
                                      space="PSUM") as ps4s, \
                         tc.tile_pool(name="ps4o", bufs=2,
                                      space="PSUM") as ps4o:
                        masks = amask_pool.tile([P, 4, 512], F32)
                        for ridx in range(4):
                            nc.gpsimd.memset(masks[:, ridx, :], 1.0)
                            # keep (=1.0) where f - p - ridx*128 >= 0,
                            # i.e. s = ridx*128+p <= t = f  (is_le is
                            # unsupported in walrus affine_select codegen)
                            nc.gpsimd.affine_select(
                                out=masks[:, ridx, :],
                                in_=masks[:, ridx, :],
                                compare_op=AL.is_ge, fill=0.0,
                                base=-(ridx * P), channel_multiplier=-1,
                                pattern=[[1, 512]])

                        ffw_s = ph4b.tile([P, E], F32R, tag="ffw")
                        nc.sync.dma_start(ffw_s[:], ffw[:, :])
                        for b in range(4):
                            for j in range(2):
                                t0 = b * 1024 + j * 512
                                i_max = 4 * j + 3
                                for hh in range(2):
                                    hsl = slice(64 * hh, 64 * hh + 64)
                                    vsl = slice(65 * hh, 65 * hh + 65)
                                    exps = []
                                    for i in range(i_max + 1):
                                        s0 = b * 1024 + i * P
                                        pss = ps4s.tile([P, 512], F32,
                                                        tag="pss")
                                        nc.tensor.matmul(
                                            pss[:],
                                            lhsT=kT[hsl, s0:s0 + P],
                                            rhs=qT[hsl, t0:t0 + 512],
                                            start=True, stop=True)
                                        ex = ph4.tile([P, 512], F32R,
                                                      tag="ex")
                                        nc.scalar.activation(
                                            ex[:], pss[:], ACT.Exp,
                                            scale=float(HS ** 0.5))
                                        r = i * P - j * 512
                                        if r >= 0:
                                            nc.vector.tensor_tensor(
                                                out=ex[:], in0=ex[:],
                                                in1=masks[:, r // P, :],
                                                op=AL.mult)
                                        exps.append(ex)
                                    pso = ps4o.tile([65, 512], F32,
                                                    tag="pso")
                                    for i in range(i_max + 1):
                                        g = b * 8 + i
                                        nc.tensor.matmul(
                                            pso[:], lhsT=vt[:, g, vsl],
                                            rhs=exps[i][:],
                                            start=(i == 0),
                                            stop=(i == i_max))
                                    rec = ph4b.tile([1, 512], F32R,
                                                    tag="rec")
                                    nc.vector.reciprocal(rec[:],
                                                         pso[64:65, :])
                                    rbc = ph4b.tile([64, 512], F32R,
                                                    tag="rbc")
                                    nc.gpsimd.partition_broadcast(
                                        rbc[:], rec[:1, :])
                                    nc.vector.tensor_tensor(
                                        out=oT[hsl, t0:t0 + 512],
                                        in0=pso[0:64, :], in1=rbc[:],
                                        op=AL.mult)
                                # FF partial for this finished token chunk,
                                # then its AllReduce -- the collectives
                                # pipeline behind attention on the CC core.
                                jj = b * 2 + j
                                for m in range(8):
                                    ps = ps4s.tile([P, 512], F32, tag="pss")
                                    nc.tensor.matmul(
                                        ps[:],
                                        lhsT=ffw_s[:, m * P:(m + 1) * P],
                                        rhs=oT[:, t0:t0 + 512],
                                        start=True, stop=True)
                                    st = ph4.tile([P, 512], F32, tag="st")
                                    nc.vector.tensor_copy(st[:], ps[:])
                                    nc.sync.dma_start(
                                        ffpart[jj, m * P:(m + 1) * P, :],
                                        st[:])
                                nc.gpsimd.collective_compute(
                                    "AllReduce", AL.add,
                                    replica_groups=[list(range(NC))],
                                    ins=[ffpart[jj, :, :].opt()],
                                    outs=[ffred[jj, :, :].opt()],
                                )

            # ---- phase 6: relu(ff) + fc + exp-sums --------------------
            with tc.tile_pool(name="ffT", bufs=1) as ffT_pool, \
                 tc.tile_pool(name="ph6w", bufs=2) as ph6w, \
                 tc.tile_pool(name="ph6", bufs=4) as ph6, \
                 tc.tile_pool(name="ph6s", bufs=1) as ph6s, \
                 tc.tile_pool(name="ps6", bufs=8, space="PSUM") as ps6:
                ffT = ffT_pool.tile([P, 8, BT], F32R)
                ffb_s = ph6s.tile([P, 8], F32)
                nc.sync.dma_start(ffb_s[:], ffb[:, :].rearrange("m p -> p m"))
                fcb_s = ph6s.tile([1, VS], F32)
                nc.sync.dma_start(fcb_s[:], fcb[:, :])
                es = ph6s.tile([P, 32, 8], F32)
                esf = ph6s.tile([P, 32], F32)

                for jj in range(8):
                    csl = slice(jj * 512, (jj + 1) * 512)
                    for k in range(8):
                        nc.sync.dma_start(ffT[:, k, csl].bitcast(F32),
                                          ffred[jj, k * P:(k + 1) * P, :])
                        nc.scalar.activation(ffT[:, k, csl],
                                             ffT[:, k, csl].bitcast(F32),
                                             ACT.Relu,
                                             bias=ffb_s[:, k:k + 1])

                for n in range(8):
                    nsl = slice(n * VC, (n + 1) * VC)
                    fcw_t = ph6w.tile([P, 8, VC], F32R, tag="fcw")
                    nc.sync.dma_start(
                        fcw_t[:],
                        fcw[:, nsl].rearrange("(k p) v -> p k v", p=P))
                    bbc = ph6w.tile([P, VC], F32, tag="bbc")
                    nc.gpsimd.partition_broadcast(bbc[:], fcb_s[:1, nsl])
                    for mc in range(8):
                      for mt in range(4):
                        m = mc * 4 + mt
                        ps = ps6.tile([P, VC], F32, tag="ps")
                        for k in range(8):
                            nc.tensor.matmul(
                                ps[:], lhsT=ffT[:, k, m * P:(m + 1) * P],
                                rhs=fcw_t[:, k, :],
                                start=(k == 0), stop=(k == 7))
                        lg = ph6.tile([P, VC], F32, tag="lg")
                        nc.vector.tensor_tensor(lg[:], ps[:], bbc[:],
                                                op=AL.add)
                        nc.sync.dma_start(
                            logits[m * P:(m + 1) * P, nsl], lg[:])
                        ex = ph6.tile([P, VC], F32, tag="exl")
                        nc.scalar.activation(ex[:], lg[:], ACT.Exp,
                                             accum_out=es[:, m, n:n + 1])

                for m in range(32):
                    nc.vector.reduce_sum(out=esf[:, m:m + 1],
                                         in_=es[:, m, :],
                                         axis=mybir.AxisListType.X)
                nc.sync.dma_start(
                    esum[:, :].rearrange("m p -> p m"), esf[:])

    nc.compile()
    return nc


def _get_nc():
    if "nc" not in _cache:
        _cache["nc"] = _build()
    return _cache["nc"]


def kernel(**inputs):
    global LAST_RESULT
    from concourse.bass_utils import run_bass_kernel_spmd

    x = np.asarray(inputs["x"]).reshape(BT, 1).astype(np.int32)
    y = np.asarray(inputs["y"]).reshape(BT).astype(np.int64)
    tok_emb = np.ascontiguousarray(np.asarray(inputs["tok_emb"], np.float32))
    pos_emb = np.ascontiguousarray(np.asarray(inputs["pos_emb"], np.float32))
    Wq = np.asarray(inputs["Wq"], np.float32)
    Wk = np.asarray(inputs["Wk"], np.float32)
    Wv = np.asarray(inputs["Wv"], np.float32)
    ff_w = np.asarray(inputs["ff_w"], np.float32)
    ff_b = np.asarray(inputs["ff_b"], np.float32)
    fc_w = np.asarray(inputs["fc_w"], np.float32)
    fc_b = np.asarray(inputs["fc_b"], np.float32)

    def wslice(W, c):
        # heads {2c, 2c+1}: [2, E, HS] -> [E, 2*HS]
        return np.ascontiguousarray(
            W[2 * c:2 * c + 2].transpose(1, 0, 2).reshape(E, DH))

    in_maps = []
    for c in range(NC):
        in_maps.append({
            "xidx": x,
            "temb": tok_emb,
            "pemb": pos_emb,
            "wq": wslice(Wq, c),
            "wk": wslice(Wk, c),
            "wv": wslice(Wv, c),
            "ffw": np.ascontiguousarray(ff_w[c * DH:(c + 1) * DH, :]),
            "ffb": np.ascontiguousarray(ff_b.reshape(8, P)),
            "fcw": np.ascontiguousarray(fc_w[:, c * VS:(c + 1) * VS]),
            "fcb": np.ascontiguousarray(fc_b[c * VS:(c + 1) * VS])[None, :],
        })

    nc = _get_nc()
    res = run_bass_kernel_spmd(nc, in_maps, core_ids=list(range(NC)))
    LAST_RESULT = res

    logits_full = np.concatenate(
        [res.results[c]["logits"] for c in range(NC)], axis=1)
    S = np.zeros(BT, np.float64)
    for c in range(NC):
        S += res.results[c]["esum"].reshape(BT).astype(np.float64)
    lse = np.log(S)
    tgt = logits_full[np.arange(BT), y].astype(np.float64)
    loss = np.float32(np.mean(lse - tgt))
    return logits_full, loss
